# revision 7
# baseline (speedup 1.0000x reference)
"""Batched dynamic embedding table forward (gather + bag-sum pooling) on 8 trn2 cores.

Strategy: data-parallel over the batch. The [2097152, 64] f32 table is
replicated to every core's HBM; core c handles samples [c*2048, (c+1)*2048).
For each 128-sample tile, the 50 bag slots are gathered with 50 indirect
DMAs (one index per partition — the HW-supported dma_indirect1d form),
landing in 50 column slots of a [128, 3200] SBUF tile; a single strided
DVE tensor_reduce pools the bags; results DMA back as [2048, 64] per core.
Host concatenates the 8 core outputs.

The 800 indirect DMAs per core are round-robined across all 4 SWDGE
queues (4 GpSimd Q7 worker pairs), overlapping descriptor generation.
This sits at the SWDGE descriptor-generation floor: the Q7 indirect-DMA
ucode supports exactly ONE offset per partition per instruction (128
descriptors); multi-column offset APs were HW-probed and degenerate
(the ucode emits one big contiguous-run descriptor per partition for a
2D out AP, and stops after ~one offset column for a 3D out AP), so
descriptor-generation cost per gathered row cannot be batched further
on this path. 800 instrs x ~1.3us SWDGE time / 4 queues ~= 260 us.

Execution goes through the same bass2jax/PJRT machinery as
bass_utils.run_bass_kernel_spmd (the axon path), but with `values` marked
replicated in the shard_map spec so it is staged to the devices once
instead of being concatenated 8x (saves ~3.5 GB of host->device traffic).
Falls back to bass_utils.run_bass_kernel_spmd if anything goes wrong.
"""

from contextlib import contextmanager

import numpy as np

import concourse.bass as bass
import concourse.mybir as mybir
import concourse.tile as tile
from concourse import bacc
from concourse.bass import IndirectOffsetOnAxis

CAPACITY = 2097152
DIM = 64
BATCH = 16384
BAG = 50
N_CORES = 8
P = 128

SAMPLES_PER_CORE = BATCH // N_CORES  # 2048


# Scoped patch: indirect_dma_start hardcodes queue="qPoolDynamic"; route
# each emitted InstDMACopy to the queue named in _Q_OVERRIDE instead so the
# 800 gathers spread over all 4 SWDGE queues (4 GpSimd Q7 worker pairs).
_Q_OVERRIDE = [None]
_ORIG_INSTDMA = mybir.InstDMACopy


def _patched_instdma(*a, **kw):
    if _Q_OVERRIDE[0] and kw.get("queue") == "qPoolDynamic":
        kw["queue"] = _Q_OVERRIDE[0]
    return _ORIG_INSTDMA(*a, **kw)


@contextmanager
def _queue_patch():
    mybir.InstDMACopy = _patched_instdma
    try:
        yield
    finally:
        mybir.InstDMACopy = _ORIG_INSTDMA


def build_nc(
    capacity=CAPACITY,
    dim=DIM,
    samples=SAMPLES_PER_CORE,
    bag=BAG,
    n_cores=N_CORES,
    n_queues=4,
    repeats=None,
):
    """Build the per-core Bass program (SPMD: same program, per-core idx data).

    repeats: when set, wrap the whole per-core workload in a
    tc.For_i(0, repeats) hardware loop (used by the timing harness).
    """
    assert samples % P == 0
    n_tiles = samples // P

    nc = bacc.Bacc(
        "TRN2",
        target_bir_lowering=False,
        debug=False,
        num_devices=n_cores,
        num_swdge_queues=n_queues,
    )
    values = nc.dram_tensor(
        "values", [capacity, dim], mybir.dt.float32, kind="ExternalInput"
    ).ap()
    idx = nc.dram_tensor(
        "idx", [samples, bag], mybir.dt.int32, kind="ExternalInput"
    ).ap()
    out = nc.dram_tensor(
        "out", [samples, dim], mybir.dt.float32, kind="ExternalOutput"
    ).ap()

    qi = [0]

    def body(tc, gpool, ipool, opool):
        for t in range(n_tiles):
            rows = slice(t * P, (t + 1) * P)

            idx_tile = ipool.tile([P, bag], mybir.dt.int32)
            nc.sync.dma_start(out=idx_tile[:], in_=idx[rows, :])

            g = gpool.tile([P, bag * dim], mybir.dt.float32)
            for j in range(bag):
                if n_queues > 1:
                    _Q_OVERRIDE[0] = f"qPoolDynamic{(qi[0] % n_queues) or ''}"
                    qi[0] += 1
                nc.gpsimd.indirect_dma_start(
                    out=g[:, j * dim : (j + 1) * dim],
                    out_offset=None,
                    in_=values[:],
                    in_offset=IndirectOffsetOnAxis(
                        ap=idx_tile[:, j : j + 1], axis=0
                    ),
                )
                _Q_OVERRIDE[0] = None

            o = opool.tile([P, dim], mybir.dt.float32)
            nc.vector.tensor_reduce(
                out=o[:],
                in_=g[:].rearrange("p (j d) -> p d j", d=dim),
                axis=mybir.AxisListType.X,
                op=mybir.AluOpType.add,
            )
            nc.sync.dma_start(out=out[rows, :], in_=o[:])

    with _queue_patch(), tile.TileContext(nc) as tc:
        with (
            tc.tile_pool(name="gather", bufs=3) as gpool,
            tc.tile_pool(name="idx", bufs=3) as ipool,
            tc.tile_pool(name="out", bufs=3) as opool,
        ):
            if repeats is None:
                body(tc, gpool, ipool, opool)
            else:
                with tc.For_i(0, repeats) as _i:
                    body(tc, gpool, ipool, opool)

    nc.compile()
    return nc


# ---------------------------------------------------------------------------
# Execution: bass2jax/PJRT with `values` replicated across the 8 cores.


class _Runner:
    def __init__(self, nc, n_cores, replicated=("values",)):
        import jax
        from jax.sharding import Mesh, PartitionSpec

        from concourse.bass2jax import (
            _bass_exec_p,
            install_neuronx_cc_hook,
            partition_id_tensor,
        )

        try:
            from jax.experimental.shard_map import shard_map
        except ImportError:
            shard_map = jax.shard_map

        install_neuronx_cc_hook()
        assert nc.dbg_addr is None
        self.jax = jax
        self.PartitionSpec = PartitionSpec
        self.nc = nc
        self.n_cores = n_cores
        self.replicated = set(replicated)
        partition_name = (
            nc.partition_id_tensor.name if nc.partition_id_tensor else None
        )

        in_names, out_names, out_avals = [], [], []
        for alloc in nc.m.functions[0].allocations:
            if not isinstance(alloc, mybir.MemoryLocationSet):
                continue
            name = alloc.memorylocations[0].name
            if alloc.kind == "ExternalInput":
                if name != partition_name:
                    in_names.append(name)
            elif alloc.kind == "ExternalOutput":
                out_names.append(name)
                out_avals.append(
                    jax.core.ShapedArray(
                        tuple(alloc.tensor_shape), mybir.dt.np(alloc.dtype)
                    )
                )
        self.in_names, self.out_names, self.out_avals = in_names, out_names, out_avals
        n_params = len(in_names)
        bind_names = in_names + out_names
        if partition_name is not None:
            bind_names = bind_names + [partition_name]

        def _body(*args):
            operands = list(args)
            if partition_name is not None:
                operands.append(partition_id_tensor())
            outs = _bass_exec_p.bind(
                *operands,
                out_avals=tuple(out_avals),
                in_names=tuple(bind_names),
                out_names=tuple(out_names),
                lowering_input_output_aliases=(),
                sim_require_finite=True,
                sim_require_nnan=True,
                nc=nc,
            )
            return tuple(outs)

        devices = jax.devices()[:n_cores]
        assert len(devices) >= n_cores
        self.mesh = Mesh(np.asarray(devices), ("core",))
        in_specs = tuple(
            PartitionSpec() if nm in self.replicated else PartitionSpec("core")
            for nm in in_names + out_names
        )
        out_specs = (PartitionSpec("core"),) * len(out_names)
        donate = tuple(range(n_params, n_params + len(out_names)))
        self.fn = jax.jit(
            shard_map(
                _body,
                mesh=self.mesh,
                in_specs=in_specs,
                out_specs=out_specs,
                check_rep=False,
            ),
            donate_argnums=donate,
            keep_unused=True,
        )

    def put_inputs(self, in_maps):
        from jax.sharding import NamedSharding, PartitionSpec

        args = []
        for nm in self.in_names:
            if nm in self.replicated:
                arr = np.asarray(in_maps[0][nm])
                sh = NamedSharding(self.mesh, PartitionSpec())
            else:
                arr = np.concatenate([np.asarray(m[nm]) for m in in_maps], axis=0)
                sh = NamedSharding(self.mesh, PartitionSpec("core"))
            args.append(self.jax.device_put(arr, sh))
        return args

    def _zeros(self):
        from jax.sharding import NamedSharding, PartitionSpec

        outs = []
        for av in self.out_avals:
            z = np.zeros((self.n_cores * av.shape[0], *av.shape[1:]), av.dtype)
            outs.append(
                self.jax.device_put(
                    z, NamedSharding(self.mesh, PartitionSpec("core"))
                )
            )
        return outs

    def run(self, dev_args):
        outs = self.fn(*dev_args, *self._zeros())
        return [np.asarray(o) for o in outs]


_CACHE = {}


def _make_in_maps(values, indices):
    values = np.ascontiguousarray(np.asarray(values, dtype=np.float32))
    idx_all = np.asarray(indices).reshape(BATCH, BAG).astype(np.int32)
    in_maps = []
    for c in range(N_CORES):
        sl = idx_all[c * SAMPLES_PER_CORE : (c + 1) * SAMPLES_PER_CORE]
        in_maps.append({"values": values, "idx": np.ascontiguousarray(sl)})
    return in_maps


def _get_nc():
    if "nc" not in _CACHE:
        _CACHE["nc"] = build_nc()
    return _CACHE["nc"]


def run_on_hw(values, indices):
    in_maps = _make_in_maps(values, indices)
    nc = _get_nc()
    if "runner" not in _CACHE:
        _CACHE["runner"] = _Runner(nc, N_CORES)
    r = _CACHE["runner"]
    dev = r.put_inputs(in_maps)
    outs = r.run(dev)
    # outs[i] is the concatenated [N_CORES*2048, 64] array for output i
    out_idx = r.out_names.index("out")
    return outs[out_idx].reshape(BATCH, DIM)


def kernel(values, indices):
    try:
        return run_on_hw(values, indices)
    except Exception:
        import traceback

        traceback.print_exc()
        from concourse.bass_utils import run_bass_kernel_spmd

        nc = _get_nc()
        res = run_bass_kernel_spmd(
            nc,
            _make_in_maps(values, indices),
            core_ids=list(range(N_CORES)),
        )
        return np.concatenate([r["out"] for r in res.results], axis=0)


# revision 9
# speedup vs baseline: 1.0076x; 1.0076x over previous
"""Batched dynamic embedding table forward (gather + bag-sum pooling) on 8 trn2 cores.

Strategy: data-parallel over the batch. The [2097152, 64] f32 table is
replicated to every core's HBM; core c handles samples [c*2048, (c+1)*2048).
For each 128-sample tile, the 50 bag slots are gathered with 50 indirect
DMAs (one index per partition — the HW-supported dma_indirect1d form),
landing in 50 column slots of a [128, 3200] SBUF tile; a single strided
DVE tensor_reduce pools the bags; results DMA back as [2048, 64] per core.
Host concatenates the 8 core outputs.

The 800 indirect DMAs per core are round-robined across all 4 SWDGE
queues (4 GpSimd Q7 worker pairs), overlapping descriptor generation.
This sits at the SWDGE descriptor-generation floor: the Q7 indirect-DMA
ucode supports exactly ONE offset per partition per instruction (128
descriptors); multi-column offset APs were HW-probed and degenerate
(the ucode emits one big contiguous-run descriptor per partition for a
2D out AP, and stops after ~one offset column for a 3D out AP), so
descriptor-generation cost per gathered row cannot be batched further
on this path. 800 instrs x ~1.3us SWDGE time / 4 queues ~= 260 us.

Execution goes through the same bass2jax/PJRT machinery as
bass_utils.run_bass_kernel_spmd (the axon path), but with `values` marked
replicated in the shard_map spec so it is staged to the devices once
instead of being concatenated 8x (saves ~3.5 GB of host->device traffic).
Falls back to bass_utils.run_bass_kernel_spmd if anything goes wrong.
"""

from contextlib import contextmanager

import numpy as np

import concourse.bass as bass
import concourse.mybir as mybir
import concourse.tile as tile
from concourse import bacc
from concourse.bass import IndirectOffsetOnAxis

CAPACITY = 2097152
DIM = 64
BATCH = 16384
BAG = 50
N_CORES = 8
P = 128

SAMPLES_PER_CORE = BATCH // N_CORES  # 2048


# Scoped patch: indirect_dma_start hardcodes queue="qPoolDynamic"; route
# each emitted InstDMACopy to the queue named in _Q_OVERRIDE instead so the
# 800 gathers spread over all 4 SWDGE queues (4 GpSimd Q7 worker pairs).
_Q_OVERRIDE = [None]
_ORIG_INSTDMA = mybir.InstDMACopy


def _patched_instdma(*a, **kw):
    if _Q_OVERRIDE[0] and kw.get("queue") == "qPoolDynamic":
        kw["queue"] = _Q_OVERRIDE[0]
    return _ORIG_INSTDMA(*a, **kw)


@contextmanager
def _queue_patch():
    mybir.InstDMACopy = _patched_instdma
    try:
        yield
    finally:
        mybir.InstDMACopy = _ORIG_INSTDMA


def build_nc(
    capacity=CAPACITY,
    dim=DIM,
    samples=SAMPLES_PER_CORE,
    bag=BAG,
    n_cores=N_CORES,
    n_queues=4,
    repeats=None,
    bufs=3,
):
    """Build the per-core Bass program (SPMD: same program, per-core idx data).

    repeats: when set, wrap the whole per-core workload in a
    tc.For_i(0, repeats) hardware loop (used by the timing harness).
    """
    assert samples % P == 0
    n_tiles = samples // P

    nc = bacc.Bacc(
        "TRN2",
        target_bir_lowering=False,
        debug=False,
        num_devices=n_cores,
        num_swdge_queues=n_queues,
    )
    values = nc.dram_tensor(
        "values", [capacity, dim], mybir.dt.float32, kind="ExternalInput"
    ).ap()
    idx = nc.dram_tensor(
        "idx", [samples, bag], mybir.dt.int32, kind="ExternalInput"
    ).ap()
    out = nc.dram_tensor(
        "out", [samples, dim], mybir.dt.float32, kind="ExternalOutput"
    ).ap()

    qi = [0]

    def body(tc, gpool, ipool, opool):
        for t in range(n_tiles):
            rows = slice(t * P, (t + 1) * P)

            idx_tile = ipool.tile([P, bag], mybir.dt.int32)
            nc.sync.dma_start(out=idx_tile[:], in_=idx[rows, :])

            g = gpool.tile([P, bag * dim], mybir.dt.float32)
            for j in range(bag):
                if n_queues > 1:
                    _Q_OVERRIDE[0] = f"qPoolDynamic{(qi[0] % n_queues) or ''}"
                    qi[0] += 1
                nc.gpsimd.indirect_dma_start(
                    out=g[:, j * dim : (j + 1) * dim],
                    out_offset=None,
                    in_=values[:],
                    in_offset=IndirectOffsetOnAxis(
                        ap=idx_tile[:, j : j + 1], axis=0
                    ),
                )
                _Q_OVERRIDE[0] = None

            o = opool.tile([P, dim], mybir.dt.float32)
            nc.vector.tensor_reduce(
                out=o[:],
                in_=g[:].rearrange("p (j d) -> p d j", d=dim),
                axis=mybir.AxisListType.X,
                op=mybir.AluOpType.add,
            )
            nc.sync.dma_start(out=out[rows, :], in_=o[:])

    with _queue_patch(), tile.TileContext(nc) as tc:
        with (
            tc.tile_pool(name="gather", bufs=bufs) as gpool,
            tc.tile_pool(name="idx", bufs=bufs) as ipool,
            tc.tile_pool(name="out", bufs=bufs) as opool,
        ):
            if repeats is None:
                body(tc, gpool, ipool, opool)
            else:
                with tc.For_i(0, repeats) as _i:
                    body(tc, gpool, ipool, opool)

    nc.compile()
    return nc


# ---------------------------------------------------------------------------
# Execution: bass2jax/PJRT with `values` replicated across the 8 cores.


class _Runner:
    def __init__(self, nc, n_cores, replicated=("values",)):
        import jax
        from jax.sharding import Mesh, PartitionSpec

        from concourse.bass2jax import (
            _bass_exec_p,
            install_neuronx_cc_hook,
            partition_id_tensor,
        )

        try:
            from jax.experimental.shard_map import shard_map
        except ImportError:
            shard_map = jax.shard_map

        install_neuronx_cc_hook()
        assert nc.dbg_addr is None
        self.jax = jax
        self.PartitionSpec = PartitionSpec
        self.nc = nc
        self.n_cores = n_cores
        self.replicated = set(replicated)
        partition_name = (
            nc.partition_id_tensor.name if nc.partition_id_tensor else None
        )

        in_names, out_names, out_avals = [], [], []
        for alloc in nc.m.functions[0].allocations:
            if not isinstance(alloc, mybir.MemoryLocationSet):
                continue
            name = alloc.memorylocations[0].name
            if alloc.kind == "ExternalInput":
                if name != partition_name:
                    in_names.append(name)
            elif alloc.kind == "ExternalOutput":
                out_names.append(name)
                out_avals.append(
                    jax.core.ShapedArray(
                        tuple(alloc.tensor_shape), mybir.dt.np(alloc.dtype)
                    )
                )
        self.in_names, self.out_names, self.out_avals = in_names, out_names, out_avals
        n_params = len(in_names)
        bind_names = in_names + out_names
        if partition_name is not None:
            bind_names = bind_names + [partition_name]

        def _body(*args):
            operands = list(args)
            if partition_name is not None:
                operands.append(partition_id_tensor())
            outs = _bass_exec_p.bind(
                *operands,
                out_avals=tuple(out_avals),
                in_names=tuple(bind_names),
                out_names=tuple(out_names),
                lowering_input_output_aliases=(),
                sim_require_finite=True,
                sim_require_nnan=True,
                nc=nc,
            )
            return tuple(outs)

        devices = jax.devices()[:n_cores]
        assert len(devices) >= n_cores
        self.mesh = Mesh(np.asarray(devices), ("core",))
        in_specs = tuple(
            PartitionSpec() if nm in self.replicated else PartitionSpec("core")
            for nm in in_names + out_names
        )
        out_specs = (PartitionSpec("core"),) * len(out_names)
        donate = tuple(range(n_params, n_params + len(out_names)))
        self.fn = jax.jit(
            shard_map(
                _body,
                mesh=self.mesh,
                in_specs=in_specs,
                out_specs=out_specs,
                check_rep=False,
            ),
            donate_argnums=donate,
            keep_unused=True,
        )

    def put_inputs(self, in_maps):
        from jax.sharding import NamedSharding, PartitionSpec

        args = []
        for nm in self.in_names:
            if nm in self.replicated:
                arr = np.asarray(in_maps[0][nm])
                sh = NamedSharding(self.mesh, PartitionSpec())
            else:
                arr = np.concatenate([np.asarray(m[nm]) for m in in_maps], axis=0)
                sh = NamedSharding(self.mesh, PartitionSpec("core"))
            args.append(self.jax.device_put(arr, sh))
        return args

    def _zeros(self):
        from jax.sharding import NamedSharding, PartitionSpec

        outs = []
        for av in self.out_avals:
            z = np.zeros((self.n_cores * av.shape[0], *av.shape[1:]), av.dtype)
            outs.append(
                self.jax.device_put(
                    z, NamedSharding(self.mesh, PartitionSpec("core"))
                )
            )
        return outs

    def run(self, dev_args):
        outs = self.fn(*dev_args, *self._zeros())
        return [np.asarray(o) for o in outs]


_CACHE = {}


def _make_in_maps(values, indices):
    values = np.ascontiguousarray(np.asarray(values, dtype=np.float32))
    idx_all = np.asarray(indices).reshape(BATCH, BAG).astype(np.int32)
    in_maps = []
    for c in range(N_CORES):
        sl = idx_all[c * SAMPLES_PER_CORE : (c + 1) * SAMPLES_PER_CORE]
        in_maps.append({"values": values, "idx": np.ascontiguousarray(sl)})
    return in_maps


def _get_nc():
    if "nc" not in _CACHE:
        _CACHE["nc"] = build_nc()
    return _CACHE["nc"]


def run_on_hw(values, indices):
    in_maps = _make_in_maps(values, indices)
    nc = _get_nc()
    if "runner" not in _CACHE:
        _CACHE["runner"] = _Runner(nc, N_CORES)
    r = _CACHE["runner"]
    dev = r.put_inputs(in_maps)
    outs = r.run(dev)
    # outs[i] is the concatenated [N_CORES*2048, 64] array for output i
    out_idx = r.out_names.index("out")
    return outs[out_idx].reshape(BATCH, DIM)


def kernel(values, indices):
    try:
        return run_on_hw(values, indices)
    except Exception:
        import traceback

        traceback.print_exc()
        from concourse.bass_utils import run_bass_kernel_spmd

        nc = _get_nc()
        res = run_bass_kernel_spmd(
            nc,
            _make_in_maps(values, indices),
            core_ids=list(range(N_CORES)),
        )
        return np.concatenate([r["out"] for r in res.results], axis=0)
